# revision 67
# baseline (speedup 1.0000x reference)
"""Bass/Trainium2 kernel for nn_BQAVariant (BQA: basis-weighted KV attention).

Reference computation (B=2, T=2048, D=768, H=12 q-heads, KH=4 KV basis
heads, HD=64):
  q = x@wq; k_basis = x@wk; v_basis = x@wv
  w = softmax(alpha); k/v = einsum('hj,btjd->bthd', w, {k,v}_basis)
  q,k = rmsnorm(rope(q,k)) * 1.2
  y = causal_sdpa(q, k, v, scale=HD**-0.5) @ wo

Sharding: 24 (batch, head) pairs over 8 cores -> core c handles batch c//4
and heads {3g, 3g+1, 3g+2} with g = c%4.  The alpha-softmax basis
combination is folded on the host into effective per-head wk/wv, so each
core runs three independent standard attention heads and emits its
partial c_proj output (transposed, [768, 2048]); the host sums the 4
partials per batch.

v2 kernel structure (per core), fp32/f32r data, fp32 PSUM:
  - Emission is software-pipelined and qc-major: for each 512-wide
    q-chunk, project 4 T-tiles (QKV + RoPE + RMSNorm + PE-transpose),
    then run transposed flash attention + c_proj for the chunk.  This
    keeps the PE continuously fed (pstate ramp) and overlaps stage-A
    DVE/ACT work with attention matmuls.
  - RMSNorm uses rope's norm-invariance: sumsq is computed on GPSIMD
    (fused square+accum) from the roped values, and
    rstd = exp(-0.5*ln(ms)) on the scalar engine so the only activation
    tables needed kernel-wide are {ln, exp, copy, square} (one table
    family -> no ACT_TABLE_LOAD thrash; no Sqrt anywhere).
  - Softmax denominator comes free from a ones-column appended to V
    (V' = [V | 1]); its reciprocal uses reciprocal_approx_fast (~5x
    faster than the exact DVE reciprocal), broadcast across partitions
    with a K=1 ones-matmul, one DVE multiply to normalize.
  - PSUM static layout (8 banks): psqk x2 (shared with c_proj pp),
    psv x1 (shared with k-transposes and the bcast pb), ptr x1
    (q-transposes), S^T scores [128,4,512] x1 (4 banks, k-tile j uses
    slot j%4 so scores for pair p+1 land while pair p is exp'd), and
    PV accumulator [65,2,512] x1 (2 banks, head h uses slot h%2 so the
    next head's PV chain starts while the previous head normalizes).
  - c_proj packs heads 0,1 into one [128, T] Y^T operand so the K=192
    contraction is 2 matmuls (128+64) instead of 3x64.
"""

import sys

sys.path.insert(0, "/opt/trn_rl_repo")

import numpy as np

import concourse.bass as bass
import concourse.tile as tile
from concourse import bacc, mybir
from concourse.bass import ts
from concourse.bass_utils import run_bass_kernel_spmd
from concourse.dve_ops import RECIP_APPROX_FAST_CONSTS, RECIPROCAL_APPROX_FAST
from concourse.masks import make_identity

F32 = mybir.dt.float32
F32R = mybir.dt.float32r
BF16 = mybir.dt.bfloat16

B, T, D = 2, 2048, 768
H, KH, HD = 12, 4, 64
HPC = 3            # heads per core
NCORES = 8
EPS = 1e-6
QK = 1.2
NTT = T // 128     # 16 T-tiles
NKD = D // 128     # 6 contraction tiles for projections
QCH = 512          # q-chunk width in flash stage
NQC = T // QCH     # 4 q-chunks
WQKV = 640         # 192 q + 192 k + 192 v + 64 pad

MM_DT = F32R

REPEAT = 1


def _patch_act_tables():
    """Force every activation in this kernel (Exp/Ln/Copy/Square/Identity)
    to resolve to the single `natural_log_exp_and_others` table set, so the
    scalar engine never reloads activation tables mid-kernel.  The act-set
    ids are positions in act_info.json's list, so we keep the dict intact
    and only strip our functions from the OTHER sets."""
    from concourse import bacc as _bacc

    orig = _bacc.get_activation_tables
    if getattr(orig, "_bqa_patched", False):
        return
    F = mybir.ActivationFunctionType
    mine = {F.Exp, F.Ln, F.Copy, F.Square, F.Identity}

    def patched(arch):
        t = dict(orig(arch))
        if "natural_log_exp_and_others" in t and mine <= t[
                "natural_log_exp_and_others"]:
            for name in t:
                if name != "natural_log_exp_and_others":
                    t[name] = t[name] - mine
        return t

    patched._bqa_patched = True
    _bacc.get_activation_tables = patched


def build_nc():
    _patch_act_tables()
    nc = bacc.Bacc(None, target_bir_lowering=False)

    xT = nc.declare_dram_parameter("xT", [D, T], BF16, isOutput=False)
    wqkv = nc.declare_dram_parameter("wqkv", [D, WQKV], BF16, isOutput=False)
    wo01 = nc.declare_dram_parameter("wo01", [128, D], BF16, isOutput=False)
    wo2 = nc.declare_dram_parameter("wo2", [64, D], BF16, isOutput=False)
    csn = nc.declare_dram_parameter("csn", [T, HD], BF16, isOutput=False)
    scn = nc.declare_dram_parameter("scn", [T, HD], BF16, isOutput=False)
    masks = nc.declare_dram_parameter("masks", [128, 128], BF16, isOutput=False)
    outT = nc.declare_dram_parameter("outT", [D, T], F32, isOutput=True)

    with tile.TileContext(nc) as tc:
        with (
            tc.tile_pool(name="persist", bufs=1) as persist,
            tc.tile_pool(name="qkt", bufs=1) as qkt,
            tc.tile_pool(name="ropetmp", bufs=3) as ropetmp,
            tc.tile_pool(name="p_sb", bufs=4) as p_pool,
            tc.tile_pool(name="misc", bufs=3) as misc,
            tc.tile_pool(name="co_sb", bufs=3) as co_pool,
            # PSUM budget (8 banks = 16KB/partition):
            #   tag psqk x2 bufs (2 banks; rotates ps_qk / ptr_q / cproj pp)
            #   tag psv x1 (1 bank; rotates ps_v / ptr_k / bcast pb)
            #   ps_s [128,4,512] (4 banks; k-tile j uses slot j%4)
            #   po [65,512] (1 bank; head-boundary chain hidden by S-mms)
            tc.tile_pool(name="proj_ps", bufs=2, space="PSUM") as proj_ps,
            tc.tile_pool(name="aux_ps", bufs=1, space="PSUM") as aux_ps,
            tc.tile_pool(name="attn_ps", bufs=1, space="PSUM") as attn_ps,
            tc.tile_pool(name="o_ps", bufs=1, space="PSUM") as o_ps,
        ):
            # --- persistent SBUF tensors ---
            # xT loads grouped per q-chunk (all 6 contraction chunks for 4
            # T-tiles in one 3D DMA) so tile groups land in compute order:
            # group g is in SBUF after ~(g+1)*4us while consumption needs it
            # at ~g*14us.
            wqkv_sb = persist.tile([128, NKD, WQKV], BF16)
            wqkv_r = wqkv.rearrange("(k p) n -> p k n", p=128)
            xT_sb = persist.tile([128, NKD, T], BF16)
            xT_r = xT.rearrange("(k p) t -> p k t", p=128)

            csn_sb = persist.tile([128, NTT, HD], BF16)
            scn_sb = persist.tile([128, NTT, HD], BF16)
            # rope tables + first weight chunks first: post_a(0) needs csn
            # at ~5us, proj(0) needs wqkv chunks progressively
            nc.sync.dma_start(out=csn_sb, in_=csn.rearrange("(i p) d -> p i d", p=128))
            nc.sync.dma_start(out=scn_sb, in_=scn.rearrange("(i p) d -> p i d", p=128))
            for k in range(NKD):
                nc.sync.dma_start(out=wqkv_sb[:, k, :], in_=wqkv_r[:, k, :])
            for g in range(NQC):
                gsl = ts(g, QCH)
                nc.gpsimd.dma_start(out=xT_sb[:, :, gsl], in_=xT_r[:, :, gsl])

            mask_sb = persist.tile([128, 128], BF16)
            nc.sync.dma_start(out=mask_sb, in_=masks[:])

            wo01_sb = persist.tile([128, D], BF16)
            wo2_sb = persist.tile([64, D], BF16)
            nc.sync.dma_start(out=wo01_sb, in_=wo01[:, :])
            nc.sync.dma_start(out=wo2_sb, in_=wo2[:, :])

            ident = persist.tile([128, 128], BF16)
            make_identity(nc, ident)

            ones_sb = persist.tile([128, 64], F32)
            nc.vector.memset(ones_sb, 1.0)
            ones_r = persist.tile([128, 64], MM_DT)
            nc.scalar.copy(ones_r, ones_sb)

            eps_sb = persist.tile([128, 1], F32)
            nc.vector.memset(eps_sb, EPS / (QK * QK))

            # V' with ones column: [128, i, h, 65]
            vp_sb = persist.tile([128, NTT, HPC, 65], BF16)
            nc.scalar.copy(
                vp_sb[:, :, :, 64:65],
                ones_sb[:, 0:1].unsqueeze(1).broadcast_to([128, NTT, HPC, 1]))

            # Q^T / K^T: [64, head, T]; Y^T packed: heads 0,1 on partitions
            # 0:64 / 64:128 of one [128, T] tile, head 2 separate.
            qt_all = qkt.tile([64, HPC, T], BF16)
            kt_all = qkt.tile([64, HPC, T], BF16)
            yt01 = qkt.tile([128, T], BF16)
            yt2 = qkt.tile([64, T], BF16)

            # static PSUM tensors; scores are two separate pair-tiles so the
            # dependency tracker never serializes pair p+1's S-matmuls
            # against pair p's exp read (alternating A/B)
            ps_sA = attn_ps.tile([128, 2, QCH], F32, tag="ps_sA")
            ps_sB = attn_ps.tile([128, 2, QCH], F32, tag="ps_sB")
            ps_pair = (ps_sA, ps_sB)
            po = o_ps.tile([65, QCH], F32, tag="po")

            # ---------------- stage A: projections ----------------
            roped_t = {}

            def proj_a(i):
                isl = ts(i, 128)
                ps_qk = proj_ps.tile([128, 384], F32, tag="psqk", name="ps_qk")
                ps_v = aux_ps.tile([128, 256], F32, tag="psv", name="ps_v")
                for k in range(NKD):
                    lhsT = xT_sb[:, k, isl]
                    st = dict(start=(k == 0), stop=(k == NKD - 1))
                    nc.tensor.matmul(ps_qk, lhsT, wqkv_sb[:, k, 0:384], **st)
                    nc.tensor.matmul(ps_v, lhsT, wqkv_sb[:, k, 384:640], **st)

                # V -> V' (scalar engine, Copy function: f32 -> f32r bits)
                nc.scalar.copy(
                    vp_sb[:, i, :, 0:64],
                    ps_v[:, 0:192].rearrange("p (h e) -> p h e", e=64))

                # rope for q and k (per which; >3 free dims is not encodable)
                cs = csn_sb[:, i, :].rearrange("p (two e) -> p two e", two=2) \
                    .unsqueeze(2).broadcast_to([128, 2, HPC, 32])
                sc = scn_sb[:, i, :].rearrange("p (two e) -> p two e", two=2) \
                    .unsqueeze(2).broadcast_to([128, 2, HPC, 32])
                roped = ropetmp.tile([128, 2, HPC, HD], F32, tag="roped",
                                     name="roped")
                for wh in range(2):
                    ps = ps_qk[:, wh * 192:(wh + 1) * 192]
                    hv = ps.rearrange("p (h two e) -> p two h e", two=2, e=32)
                    x1 = hv[:, 0:1, :, :].broadcast_to([128, 2, HPC, 32])
                    x2 = hv[:, 1:2, :, :].broadcast_to([128, 2, HPC, 32])
                    t1 = ropetmp.tile([128, 2, HPC, 32], F32,
                                      tag=f"t1{wh}", name="t1")
                    t2 = ropetmp.tile([128, 2, HPC, 32], F32,
                                      tag=f"t2{wh}", name="t2")
                    nc.vector.tensor_mul(t1, x1, cs)
                    nc.vector.tensor_mul(t2, x2, sc)
                    rview = roped[:, wh, :, :].rearrange(
                        "p h (two e) -> p two h e", two=2, e=32)
                    nc.vector.tensor_add(rview, t1, t2)
                roped_t[i] = roped

            def post_a(i):
                isl = ts(i, 128)
                roped = roped_t.pop(i)

                # sumsq on GPSIMD (rope preserves the norm, so rmsnorm of the
                # roped value == rmsnorm computed from these sums)
                ssum = ropetmp.tile([128, 2, HPC], F32, tag="ssum", name="ssum")
                sqs = ropetmp.tile([128, 2, HPC, HD], F32, tag="sqs", name="sqs")
                nc.gpsimd.tensor_mul(sqs, roped, roped)
                nc.vector.reduce_sum(ssum, sqs, axis=mybir.AxisListType.X)

                # rstd = 1/sqrt(ms) = exp(-0.5*ln(ms)); ln+exp share the
                # activation table family with the attention exp.
                lnt = ropetmp.tile([128, 2, HPC], F32, tag="lnt", name="lnt")
                nc.scalar.activation(lnt, ssum,
                                     mybir.ActivationFunctionType.Ln,
                                     bias=eps_sb,
                                     scale=1.0 / (HD * QK * QK))
                rstd = ropetmp.tile([128, 2, HPC], F32, tag="rstd", name="rstd")
                nc.scalar.activation(rstd, lnt,
                                     mybir.ActivationFunctionType.Exp,
                                     scale=-0.5)

                normed = ropetmp.tile([128, 2, HPC, HD], BF16, tag="normed",
                                      name="normed")
                nc.vector.tensor_mul(
                    normed, roped,
                    rstd.unsqueeze(3).broadcast_to([128, 2, HPC, HD]))

                # PE transposes; q -> ptr bank (evict on scalar), k -> psv
                # bank (evict on vector)
                ptr_q = proj_ps.tile([64, HPC, 128], BF16, tag="psqk",
                                     name="ptr_q")
                for hh in range(HPC):
                    nc.tensor.transpose(ptr_q[:, hh, :], normed[:, 0, hh, :],
                                        ident)
                nc.scalar.copy(qt_all[:, :, isl], ptr_q)
                ptr_k = aux_ps.tile([64, HPC, 128], BF16, tag="psv", name="ptr_k")
                for hh in range(HPC):
                    nc.tensor.transpose(ptr_k[:, hh, :], normed[:, 1, hh, :],
                                        ident)
                nc.vector.tensor_copy(kt_all[:, :, isl], ptr_k)

            # ---------------- stage B: attention ----------------
            def finish_head(qc, h, ou):
                # normalize chain: fast reciprocal of the V'-ones row,
                # partition-broadcast via K=1 ones-matmul, one multiply
                qsl = ts(qc, QCH)
                den = misc.tile([1, QCH], F32, tag="den", name="den",
                                bufs=3)
                nc.vector.tensor_copy(den, ou[64:65, :])
                rsb32 = misc.tile([1, QCH], F32, tag="rsb32", name="rsb32")
                nc.vector.reciprocal_approx_fast(rsb32, den)
                rsb = misc.tile([1, QCH], MM_DT, tag="rsb", name="rsb")
                nc.vector.tensor_copy(rsb, rsb32)
                pb = aux_ps.tile([64, QCH], F32, tag="psv", name="pb")
                nc.tensor.matmul(pb, ones_r[0:1, :], rsb,
                                 start=True, stop=True)
                if h < 2:
                    dst = yt01[h * 64:(h + 1) * 64, qsl]
                else:
                    dst = yt2[:, qsl]
                nc.vector.tensor_mul(dst, ou[0:64, :], pb)

            def attention(qc, filler=()):
                # software-pipelined pair loop: pair p+1's S^T matmuls are
                # emitted BEFORE pair p's PV matmuls so the in-order PE
                # queue has score work to run while the scalar engine exps
                # pair p.  The pipeline runs across head boundaries too.
                # `filler` items (next-chunk stage-A closures) are drained
                # one per pair-iteration so their DVE/PE work interleaves
                # with attention instead of queuing behind its tail.
                filler = list(filler)
                qsl = ts(qc, QCH)
                njt = 4 * qc + 4
                fin = None
                pends = []        # [(h, info, p_t), ...] flushed at depth 2
                fin_queue = []    # [[ready, closure], ...]

                def emit_pv(h, info, p_t):
                    for j, idx, s, c0 in info:
                        nc.tensor.matmul(po[:, c0:QCH], vp_sb[:, j, h, :],
                                         p_t[:, idx, c0:QCH],
                                         start=(j == 0), stop=(j == njt - 1))

                def emit_evict(h):
                    # evict the unnormalized accumulator so the po bank
                    # frees for the next head immediately; the normalize
                    # chains are deferred (fin_queue / fin) so their PE
                    # broadcast-matmul never heads the in-order PE queue
                    # before its reciprocal chain is done
                    nonlocal fin
                    # ONE copy frees the po bank; the partition-0 denominator
                    # row for the custom-DVE reciprocal is re-copied lazily
                    # from SBUF inside finish_head
                    ou = misc.tile([65, QCH], F32, tag="ou", name="ou",
                                   bufs=3)
                    nc.vector.tensor_copy(ou, po)
                    clo = (lambda hh, oo: lambda: finish_head(
                        qc, hh, oo))(h, ou)
                    if h < HPC - 1:
                        fin_queue.append([False, clo])
                    else:
                        fin = clo

                pi = 0
                for h in range(HPC):
                    for j0 in range(0, njt, 2):
                        sst = ps_pair[pi]
                        pi ^= 1
                        info = []
                        for idx, j in enumerate((j0, j0 + 1)):
                            s = j - 4 * qc
                            c0 = 128 * s if s > 0 else 0
                            info.append((j, idx, s, c0))
                            nc.tensor.matmul(
                                sst[:, idx, c0:QCH],
                                kt_all[:, h, ts(j, 128)],
                                qt_all[:, h, qc * QCH + c0:(qc + 1) * QCH],
                                start=True, stop=True)
                        p_t = p_pool.tile([128, 2, QCH], BF16, tag="pt",
                                          name="p_t")
                        if all(c0 == 0 for _, _, _, c0 in info):
                            nc.scalar.activation(
                                p_t, sst,
                                mybir.ActivationFunctionType.Exp,
                                scale=float(HD) ** -0.5)
                        else:
                            for j, idx, s, c0 in info:
                                nc.scalar.activation(
                                    p_t[:, idx, c0:QCH],
                                    sst[:, idx, c0:QCH],
                                    mybir.ActivationFunctionType.Exp,
                                    scale=float(HD) ** -0.5)
                        for j, idx, s, c0 in info:
                            if s >= 0:
                                nc.gpsimd.tensor_mul(p_t[:, idx, c0:c0 + 128],
                                                     p_t[:, idx, c0:c0 + 128],
                                                     mask_sb)
                        pends.append((h, info, p_t))
                        if len(pends) > 1:
                            ph, pinfo, ppt = pends.pop(0)
                            emit_pv(ph, pinfo, ppt)
                            if ph != h:
                                emit_evict(ph)
                        # flush one matured finish-chain per iteration
                        if fin_queue and fin_queue[0][0]:
                            fin_queue.pop(0)[1]()
                        for e in fin_queue:
                            e[0] = True
                        if filler:
                            filler.pop(0)()
                while pends:
                    ph, pinfo, ppt = pends.pop(0)
                    emit_pv(ph, pinfo, ppt)
                    if not pends or pends[0][0] != ph:
                        emit_evict(ph)
                for _, clo in fin_queue:
                    clo()
                for f in filler:
                    f()
                return fin

            def cproj(qc, fin):
                # all six yt01 (heads 0,1) matmuls first — four target idle
                # ps_s banks — covering the last head's normalize chain,
                # which is emitted (fin) before the yt2 matmuls need it
                qsl = ts(qc, QCH)
                pps = {}
                for m in range(D // 128):
                    if m < 4:
                        pp = ps_pair[m // 2][:, m % 2, :]
                    else:
                        pp = proj_ps.tile([128, QCH], F32, tag="psqk",
                                          name="pp")
                    pps[m] = pp
                    nc.tensor.matmul(pp, wo01_sb[:, ts(m, 128)], yt01[:, qsl],
                                     start=True, stop=False)
                fin()
                for m in range(D // 128):
                    pp = pps.pop(m)
                    nc.tensor.matmul(pp, wo2_sb[:, ts(m, 128)], yt2[:, qsl],
                                     start=False, stop=True)
                    ot = co_pool.tile([128, QCH], F32, tag="ot", name="ot")
                    nc.vector.tensor_copy(ot, pp)
                    nc.sync.dma_start(out=outT[ts(m, 128), qsl], in_=ot)

            def emit_body():
                # software-pipelined emission: proj_a leads post_a by 2
                # tiles, and 2 tiles of the NEXT chunk's projections are
                # emitted before each attention(qc) so the PE queue never
                # drains while the stage-A post chains complete.
                state = {"p": 0, "d": 0}

                def advance_to(need_post, proj_ahead):
                    cap = min(need_post + proj_ahead, NTT)
                    while state["d"] < need_post or state["p"] < cap:
                        if state["p"] < cap and state["p"] - state["d"] <= 2:
                            proj_a(state["p"])
                            state["p"] += 1
                        elif state["d"] < state["p"]:
                            post_a(state["d"])
                            state["d"] += 1
                        else:
                            break

                def mk(fn, i, key):
                    def f():
                        fn(i)
                        state[key] = i + 1
                    return f

                for qc in range(NQC):
                    advance_to(4 * qc + 4, 2)
                    # next-chunk stage-A work rides inside attention's pair
                    # loop, keeping both PE and DVE queues supplied
                    filler = []
                    plan_p, plan_d = state["p"], state["d"]
                    for _ in range(2):
                        if plan_p < min(NTT, 4 * qc + 8):
                            filler.append(mk(proj_a, plan_p, "p"))
                            plan_p += 1
                        if plan_d < min(4 * qc + 6, NTT):
                            filler.append(mk(post_a, plan_d, "d"))
                            plan_d += 1
                    fin = attention(qc, filler)
                    # two more full tiles between attention and cproj: their
                    # PE work covers the trailing normalize chains
                    advance_to(min(4 * qc + 8, NTT), 2)
                    cproj(qc, fin)

            if REPEAT > 1:
                with tc.For_i(0, REPEAT, 1):
                    emit_body()
            else:
                emit_body()

    nc.finalize()
    return nc


_NC = None


def _get_nc():
    global _NC
    if _NC is None:
        _NC = build_nc()
    return _NC


def _prep_inputs(x, wq, wk, wv, wo, alpha, cos, sin):
    x = np.asarray(x, dtype=np.float32)
    wq = np.asarray(wq, dtype=np.float32)
    wk = np.asarray(wk, dtype=np.float32)
    wv = np.asarray(wv, dtype=np.float32)
    wo = np.asarray(wo, dtype=np.float32)
    alpha = np.asarray(alpha, dtype=np.float32)
    cos = np.asarray(cos, dtype=np.float32)
    sin = np.asarray(sin, dtype=np.float32)

    # softmax over basis heads (fp32, stable)
    a = alpha - alpha.max(axis=-1, keepdims=True)
    e = np.exp(a)
    w = e / e.sum(axis=-1, keepdims=True)          # [H, KH]

    # fold the basis combination into effective per-head wk / wv
    wk_eff = np.einsum("dje,hj->dhe", wk.reshape(D, KH, HD), w).reshape(D, H * HD)
    wv_eff = np.einsum("dje,hj->dhe", wv.reshape(D, KH, HD), w).reshape(D, H * HD)

    csn = np.ascontiguousarray(np.concatenate([cos, sin], axis=1))     # [T, 64]
    scn = np.ascontiguousarray(np.concatenate([-sin, cos], axis=1))    # [T, 64]

    # single [128, 128] triangular mask (k <= q) for diagonal sub-blocks
    kk = np.arange(128)[:, None]
    qq = np.arange(128)[None, :]
    masks = np.ascontiguousarray((kk <= qq).astype(np.float32))

    import ml_dtypes
    bf16 = ml_dtypes.bfloat16

    in_maps = []
    for c in range(NCORES):
        b, g = c // 4, c % 4
        sl = slice(g * HPC * HD, (g + 1) * HPC * HD)
        wqkv = np.zeros((D, WQKV), dtype=np.float32)
        wqkv[:, 0:192] = wq[:, sl]
        wqkv[:, 192:384] = wk_eff[:, sl]
        wqkv[:, 384:576] = wv_eff[:, sl]
        wo_c = np.ascontiguousarray(wo[sl, :])
        in_maps.append({
            "xT": np.ascontiguousarray(x[b].T).astype(bf16),
            "wqkv": wqkv.astype(bf16),
            "wo01": np.ascontiguousarray(wo_c[0:128, :]).astype(bf16),
            "wo2": np.ascontiguousarray(wo_c[128:192, :]).astype(bf16),
            "csn": csn.astype(bf16),
            "scn": scn.astype(bf16),
            "masks": masks.astype(bf16),
        })
    return in_maps


def run(trace=False, **inputs):
    nc = _get_nc()
    in_maps = _prep_inputs(**inputs)
    res = run_bass_kernel_spmd(nc, in_maps, list(range(NCORES)), trace=trace)
    out = np.zeros((B, T, D), dtype=np.float32)
    for c in range(NCORES):
        out[c // 4] += res.results[c]["outT"].T
    return out, res


def kernel(**inputs):
    out, _ = run(**inputs)
    return out


# revision 68
# speedup vs baseline: 1.1177x; 1.1177x over previous
"""Bass/Trainium2 kernel for nn_BQAVariant (BQA: basis-weighted KV attention).

Reference computation (B=2, T=2048, D=768, H=12 q-heads, KH=4 KV basis
heads, HD=64):
  q = x@wq; k_basis = x@wk; v_basis = x@wv
  w = softmax(alpha); k/v = einsum('hj,btjd->bthd', w, {k,v}_basis)
  q,k = rmsnorm(rope(q,k)) * 1.2
  y = causal_sdpa(q, k, v, scale=HD**-0.5) @ wo

Sharding: 24 (batch, head) pairs over 8 cores -> core c handles batch c//4
and heads {3g, 3g+1, 3g+2} with g = c%4.  The alpha-softmax basis
combination is folded on the host into effective per-head wk/wv, so each
core runs three independent standard attention heads and emits its
partial c_proj output (transposed, [768, 2048]); the host sums the 4
partials per batch.

v2 kernel structure (per core), fp32/f32r data, fp32 PSUM:
  - Emission is software-pipelined and qc-major: for each 512-wide
    q-chunk, project 4 T-tiles (QKV + RoPE + RMSNorm + PE-transpose),
    then run transposed flash attention + c_proj for the chunk.  This
    keeps the PE continuously fed (pstate ramp) and overlaps stage-A
    DVE/ACT work with attention matmuls.
  - RMSNorm uses rope's norm-invariance: sumsq is computed on GPSIMD
    (fused square+accum) from the roped values, and
    rstd = exp(-0.5*ln(ms)) on the scalar engine so the only activation
    tables needed kernel-wide are {ln, exp, copy, square} (one table
    family -> no ACT_TABLE_LOAD thrash; no Sqrt anywhere).
  - Softmax denominator comes free from a ones-column appended to V
    (V' = [V | 1]); its reciprocal uses reciprocal_approx_fast (~5x
    faster than the exact DVE reciprocal), broadcast across partitions
    with a K=1 ones-matmul, one DVE multiply to normalize.
  - PSUM static layout (8 banks): psqk x2 (shared with c_proj pp),
    psv x1 (shared with k-transposes and the bcast pb), ptr x1
    (q-transposes), S^T scores [128,4,512] x1 (4 banks, k-tile j uses
    slot j%4 so scores for pair p+1 land while pair p is exp'd), and
    PV accumulator [65,2,512] x1 (2 banks, head h uses slot h%2 so the
    next head's PV chain starts while the previous head normalizes).
  - c_proj packs heads 0,1 into one [128, T] Y^T operand so the K=192
    contraction is 2 matmuls (128+64) instead of 3x64.
"""

import sys

sys.path.insert(0, "/opt/trn_rl_repo")

import numpy as np

import concourse.bass as bass
import concourse.tile as tile
from concourse import bacc, mybir
from concourse.bass import ts
from concourse.bass_utils import run_bass_kernel_spmd
from concourse.dve_ops import RECIP_APPROX_FAST_CONSTS, RECIPROCAL_APPROX_FAST
from concourse.masks import make_identity

F32 = mybir.dt.float32
F32R = mybir.dt.float32r
BF16 = mybir.dt.bfloat16

B, T, D = 2, 2048, 768
H, KH, HD = 12, 4, 64
HPC = 3            # heads per core
NCORES = 8
EPS = 1e-6
QK = 1.2
NTT = T // 128     # 16 T-tiles
NKD = D // 128     # 6 contraction tiles for projections
QCH = 512          # q-chunk width in flash stage
NQC = T // QCH     # 4 q-chunks
WQKV = 640         # 192 q + 192 k + 192 v + 64 pad

MM_DT = F32R

REPEAT = 1


def _patch_act_tables():
    """Force every activation in this kernel (Exp/Ln/Copy/Square/Identity)
    to resolve to the single `natural_log_exp_and_others` table set, so the
    scalar engine never reloads activation tables mid-kernel.  The act-set
    ids are positions in act_info.json's list, so we keep the dict intact
    and only strip our functions from the OTHER sets."""
    from concourse import bacc as _bacc

    orig = _bacc.get_activation_tables
    if getattr(orig, "_bqa_patched", False):
        return
    F = mybir.ActivationFunctionType
    mine = {F.Exp, F.Ln, F.Copy, F.Square, F.Identity}

    def patched(arch):
        t = dict(orig(arch))
        if "natural_log_exp_and_others" in t and mine <= t[
                "natural_log_exp_and_others"]:
            for name in t:
                if name != "natural_log_exp_and_others":
                    t[name] = t[name] - mine
        return t

    patched._bqa_patched = True
    _bacc.get_activation_tables = patched


def build_nc():
    _patch_act_tables()
    nc = bacc.Bacc(None, target_bir_lowering=False)

    xT = nc.declare_dram_parameter("xT", [D, T], BF16, isOutput=False)
    wqkv = nc.declare_dram_parameter("wqkv", [D, WQKV], BF16, isOutput=False)
    wo01 = nc.declare_dram_parameter("wo01", [128, D], BF16, isOutput=False)
    wo2 = nc.declare_dram_parameter("wo2", [64, D], BF16, isOutput=False)
    csn = nc.declare_dram_parameter("csn", [T, HD], BF16, isOutput=False)
    scn = nc.declare_dram_parameter("scn", [T, HD], BF16, isOutput=False)
    masks = nc.declare_dram_parameter("masks", [128, 128], BF16, isOutput=False)
    outT = nc.declare_dram_parameter("outT", [D, T], F32, isOutput=True)

    with tile.TileContext(nc) as tc:
        with (
            tc.tile_pool(name="persist", bufs=1) as persist,
            tc.tile_pool(name="qkt", bufs=1) as qkt,
            tc.tile_pool(name="ropetmp", bufs=2) as ropetmp,
            tc.tile_pool(name="p_sb", bufs=3) as p_pool,
            tc.tile_pool(name="misc", bufs=2) as misc,
            tc.tile_pool(name="co_sb", bufs=2) as co_pool,
            # PSUM budget (8 banks = 16KB/partition):
            #   tag psqk x2 bufs (2 banks; rotates ps_qk / ptr_q / cproj pp)
            #   tag psv x1 (1 bank; rotates ps_v / ptr_k / bcast pb)
            #   ps_s [128,4,512] (4 banks; k-tile j uses slot j%4)
            #   po [65,512] (1 bank; head-boundary chain hidden by S-mms)
            tc.tile_pool(name="proj_ps", bufs=2, space="PSUM") as proj_ps,
            tc.tile_pool(name="aux_ps", bufs=1, space="PSUM") as aux_ps,
            tc.tile_pool(name="attn_ps", bufs=1, space="PSUM") as attn_ps,
            tc.tile_pool(name="o_ps", bufs=1, space="PSUM") as o_ps,
        ):
            # --- persistent SBUF tensors ---
            # xT loads grouped per q-chunk (all 6 contraction chunks for 4
            # T-tiles in one 3D DMA) so tile groups land in compute order:
            # group g is in SBUF after ~(g+1)*4us while consumption needs it
            # at ~g*14us.
            wqkv_sb = persist.tile([128, NKD, WQKV], BF16)
            wqkv_r = wqkv.rearrange("(k p) n -> p k n", p=128)
            xT_sb = persist.tile([128, NKD, T], BF16)
            xT_r = xT.rearrange("(k p) t -> p k t", p=128)

            csn_sb = persist.tile([128, NTT, HD], BF16)
            scn_sb = persist.tile([128, NTT, HD], BF16)
            # rope tables + first weight chunks first: post_a(0) needs csn
            # at ~5us, proj(0) needs wqkv chunks progressively
            nc.sync.dma_start(out=csn_sb, in_=csn.rearrange("(i p) d -> p i d", p=128))
            nc.sync.dma_start(out=scn_sb, in_=scn.rearrange("(i p) d -> p i d", p=128))
            for k in range(NKD):
                nc.sync.dma_start(out=wqkv_sb[:, k, :], in_=wqkv_r[:, k, :])
            for g in range(NQC):
                gsl = ts(g, QCH)
                nc.gpsimd.dma_start(out=xT_sb[:, :, gsl], in_=xT_r[:, :, gsl])

            mask_sb = persist.tile([128, 128], BF16)
            nc.sync.dma_start(out=mask_sb, in_=masks[:])

            wo01_sb = persist.tile([128, D], BF16)
            wo2_sb = persist.tile([64, D], BF16)
            nc.sync.dma_start(out=wo01_sb, in_=wo01[:, :])
            nc.sync.dma_start(out=wo2_sb, in_=wo2[:, :])

            ident = persist.tile([128, 128], BF16)
            make_identity(nc, ident)

            ones_sb = persist.tile([128, 64], F32)
            nc.vector.memset(ones_sb, 1.0)
            ones_r = persist.tile([128, 64], MM_DT)
            nc.scalar.copy(ones_r, ones_sb)

            eps_sb = persist.tile([128, 1], F32)
            nc.vector.memset(eps_sb, EPS / (QK * QK))

            # V' with ones column: [128, i, h, 65]
            vp_sb = persist.tile([128, NTT, HPC, 65], BF16)
            nc.scalar.copy(
                vp_sb[:, :, :, 64:65],
                ones_sb[:, 0:1].unsqueeze(1).broadcast_to([128, NTT, HPC, 1]))

            # Q^T / K^T: [64, head, T]; Y^T packed: heads 0,1 on partitions
            # 0:64 / 64:128 of one [128, T] tile, head 2 separate.
            qt_all = qkt.tile([64, HPC, T], BF16)
            kt_all = qkt.tile([64, HPC, T], BF16)
            yt01 = qkt.tile([128, T], BF16)
            yt2 = qkt.tile([64, T], BF16)

            # static PSUM tensors; scores are two separate pair-tiles so the
            # dependency tracker never serializes pair p+1's S-matmuls
            # against pair p's exp read (alternating A/B)
            ps_sA = attn_ps.tile([128, 2, QCH], F32, tag="ps_sA")
            ps_sB = attn_ps.tile([128, 2, QCH], F32, tag="ps_sB")
            ps_pair = (ps_sA, ps_sB)
            po = o_ps.tile([65, QCH], F32, tag="po")

            # ---------------- stage A: projections ----------------
            roped_t = {}

            def proj_a(i):
                isl = ts(i, 128)
                ps_qk = proj_ps.tile([128, 384], F32, tag="psqk", name="ps_qk")
                ps_v = aux_ps.tile([128, 256], F32, tag="psv", name="ps_v")
                for k in range(NKD):
                    lhsT = xT_sb[:, k, isl]
                    st = dict(start=(k == 0), stop=(k == NKD - 1))
                    nc.tensor.matmul(ps_qk, lhsT, wqkv_sb[:, k, 0:384], **st)
                    nc.tensor.matmul(ps_v, lhsT, wqkv_sb[:, k, 384:640], **st)

                # V -> V' (scalar engine, Copy function: f32 -> f32r bits)
                nc.scalar.copy(
                    vp_sb[:, i, :, 0:64],
                    ps_v[:, 0:192].rearrange("p (h e) -> p h e", e=64))

                # rope for q and k (per which; >3 free dims is not encodable)
                cs = csn_sb[:, i, :].rearrange("p (two e) -> p two e", two=2) \
                    .unsqueeze(2).broadcast_to([128, 2, HPC, 32])
                sc = scn_sb[:, i, :].rearrange("p (two e) -> p two e", two=2) \
                    .unsqueeze(2).broadcast_to([128, 2, HPC, 32])
                roped = ropetmp.tile([128, 2, HPC, HD], F32, tag="roped",
                                     name="roped")
                for wh in range(2):
                    ps = ps_qk[:, wh * 192:(wh + 1) * 192]
                    hv = ps.rearrange("p (h two e) -> p two h e", two=2, e=32)
                    x1 = hv[:, 0:1, :, :].broadcast_to([128, 2, HPC, 32])
                    x2 = hv[:, 1:2, :, :].broadcast_to([128, 2, HPC, 32])
                    t1 = ropetmp.tile([128, 2, HPC, 32], F32,
                                      tag=f"t1{wh}", name="t1")
                    t2 = ropetmp.tile([128, 2, HPC, 32], F32,
                                      tag=f"t2{wh}", name="t2")
                    nc.vector.tensor_mul(t1, x1, cs)
                    nc.vector.tensor_mul(t2, x2, sc)
                    rview = roped[:, wh, :, :].rearrange(
                        "p h (two e) -> p two h e", two=2, e=32)
                    nc.vector.tensor_add(rview, t1, t2)
                roped_t[i] = roped

            def post_a(i):
                isl = ts(i, 128)
                roped = roped_t.pop(i)

                # sumsq on GPSIMD (rope preserves the norm, so rmsnorm of the
                # roped value == rmsnorm computed from these sums)
                ssum = ropetmp.tile([128, 2, HPC], F32, tag="ssum", name="ssum")
                sqs = ropetmp.tile([128, 2, HPC, HD], F32, tag="sqs", name="sqs")
                nc.gpsimd.tensor_mul(sqs, roped, roped)
                nc.vector.reduce_sum(ssum, sqs, axis=mybir.AxisListType.X)

                # rstd = 1/sqrt(ms) = exp(-0.5*ln(ms)); ln+exp share the
                # activation table family with the attention exp.
                lnt = ropetmp.tile([128, 2, HPC], F32, tag="lnt", name="lnt")
                nc.scalar.activation(lnt, ssum,
                                     mybir.ActivationFunctionType.Ln,
                                     bias=eps_sb,
                                     scale=1.0 / (HD * QK * QK))
                rstd = ropetmp.tile([128, 2, HPC], F32, tag="rstd", name="rstd")
                nc.scalar.activation(rstd, lnt,
                                     mybir.ActivationFunctionType.Exp,
                                     scale=-0.5)

                normed = ropetmp.tile([128, 2, HPC, HD], BF16, tag="normed",
                                      name="normed")
                nc.vector.tensor_mul(
                    normed, roped,
                    rstd.unsqueeze(3).broadcast_to([128, 2, HPC, HD]))

                # PE transposes; q -> ptr bank (evict on scalar), k -> psv
                # bank (evict on vector)
                ptr_q = proj_ps.tile([64, HPC, 128], BF16, tag="psqk",
                                     name="ptr_q")
                for hh in range(HPC):
                    nc.tensor.transpose(ptr_q[:, hh, :], normed[:, 0, hh, :],
                                        ident)
                nc.scalar.copy(qt_all[:, :, isl], ptr_q)
                ptr_k = aux_ps.tile([64, HPC, 128], BF16, tag="psv", name="ptr_k")
                for hh in range(HPC):
                    nc.tensor.transpose(ptr_k[:, hh, :], normed[:, 1, hh, :],
                                        ident)
                nc.vector.tensor_copy(kt_all[:, :, isl], ptr_k)

            # ---------------- stage B: attention ----------------
            def finish_head(qc, h, ou):
                # normalize chain: fast reciprocal of the V'-ones row,
                # partition-broadcast via K=1 ones-matmul, one multiply
                qsl = ts(qc, QCH)
                den = misc.tile([1, QCH], F32, tag="den", name="den",
                                bufs=3)
                nc.vector.tensor_copy(den, ou[64:65, :])
                rsb32 = misc.tile([1, QCH], F32, tag="rsb32", name="rsb32")
                nc.vector.reciprocal_approx_fast(rsb32, den)
                rsb = misc.tile([1, QCH], MM_DT, tag="rsb", name="rsb")
                nc.vector.tensor_copy(rsb, rsb32)
                pb = aux_ps.tile([64, QCH], F32, tag="psv", name="pb")
                nc.tensor.matmul(pb, ones_r[0:1, :], rsb,
                                 start=True, stop=True)
                if h < 2:
                    dst = yt01[h * 64:(h + 1) * 64, qsl]
                else:
                    dst = yt2[:, qsl]
                nc.vector.tensor_mul(dst, ou[0:64, :], pb)

            def attention(qc, filler=()):
                # software-pipelined pair loop: pair p+1's S^T matmuls are
                # emitted BEFORE pair p's PV matmuls so the in-order PE
                # queue has score work to run while the scalar engine exps
                # pair p.  The pipeline runs across head boundaries too.
                # `filler` items (next-chunk stage-A closures) are drained
                # one per pair-iteration so their DVE/PE work interleaves
                # with attention instead of queuing behind its tail.
                filler = list(filler)
                qsl = ts(qc, QCH)
                njt = 4 * qc + 4
                fin = None
                pends = []        # [(h, info, p_t), ...] flushed at depth 2
                fin_queue = []    # [[ready, closure], ...]

                def emit_pv(h, info, p_t):
                    for j, idx, s, c0 in info:
                        nc.tensor.matmul(po[:, c0:QCH], vp_sb[:, j, h, :],
                                         p_t[:, idx, c0:QCH],
                                         start=(j == 0), stop=(j == njt - 1))

                def emit_evict(h):
                    # evict the unnormalized accumulator so the po bank
                    # frees for the next head immediately; the normalize
                    # chains are deferred (fin_queue / fin) so their PE
                    # broadcast-matmul never heads the in-order PE queue
                    # before its reciprocal chain is done
                    nonlocal fin
                    # ONE copy frees the po bank; the partition-0 denominator
                    # row for the custom-DVE reciprocal is re-copied lazily
                    # from SBUF inside finish_head
                    ou = misc.tile([65, QCH], F32, tag="ou", name="ou",
                                   bufs=3)
                    nc.vector.tensor_copy(ou, po)
                    clo = (lambda hh, oo: lambda: finish_head(
                        qc, hh, oo))(h, ou)
                    if h < HPC - 1:
                        fin_queue.append([False, clo])
                    else:
                        fin = clo

                pi = 0
                for h in range(HPC):
                    for j0 in range(0, njt, 2):
                        sst = ps_pair[pi]
                        pi ^= 1
                        info = []
                        for idx, j in enumerate((j0, j0 + 1)):
                            s = j - 4 * qc
                            c0 = 128 * s if s > 0 else 0
                            info.append((j, idx, s, c0))
                            nc.tensor.matmul(
                                sst[:, idx, c0:QCH],
                                kt_all[:, h, ts(j, 128)],
                                qt_all[:, h, qc * QCH + c0:(qc + 1) * QCH],
                                start=True, stop=True)
                        p_t = p_pool.tile([128, 2, QCH], BF16, tag="pt",
                                          name="p_t")
                        if all(c0 == 0 for _, _, _, c0 in info):
                            nc.scalar.activation(
                                p_t, sst,
                                mybir.ActivationFunctionType.Exp,
                                scale=float(HD) ** -0.5)
                        else:
                            for j, idx, s, c0 in info:
                                nc.scalar.activation(
                                    p_t[:, idx, c0:QCH],
                                    sst[:, idx, c0:QCH],
                                    mybir.ActivationFunctionType.Exp,
                                    scale=float(HD) ** -0.5)
                        for j, idx, s, c0 in info:
                            if s >= 0:
                                nc.gpsimd.tensor_mul(p_t[:, idx, c0:c0 + 128],
                                                     p_t[:, idx, c0:c0 + 128],
                                                     mask_sb)
                        pends.append((h, info, p_t))
                        if len(pends) > 1:
                            ph, pinfo, ppt = pends.pop(0)
                            emit_pv(ph, pinfo, ppt)
                            if ph != h:
                                emit_evict(ph)
                        # flush one matured finish-chain per iteration
                        if fin_queue and fin_queue[0][0]:
                            fin_queue.pop(0)[1]()
                        for e in fin_queue:
                            e[0] = True
                        if filler:
                            filler.pop(0)()
                while pends:
                    ph, pinfo, ppt = pends.pop(0)
                    emit_pv(ph, pinfo, ppt)
                    if not pends or pends[0][0] != ph:
                        emit_evict(ph)
                for _, clo in fin_queue:
                    clo()
                for f in filler:
                    f()
                return fin

            def cproj(qc, fin):
                # all six yt01 (heads 0,1) matmuls first — four target idle
                # ps_s banks — covering the last head's normalize chain,
                # which is emitted (fin) before the yt2 matmuls need it
                qsl = ts(qc, QCH)
                pps = {}
                for m in range(D // 128):
                    if m < 4:
                        pp = ps_pair[m // 2][:, m % 2, :]
                    else:
                        pp = proj_ps.tile([128, QCH], F32, tag="psqk",
                                          name="pp")
                    pps[m] = pp
                    nc.tensor.matmul(pp, wo01_sb[:, ts(m, 128)], yt01[:, qsl],
                                     start=True, stop=False)
                fin()
                for m in range(D // 128):
                    pp = pps.pop(m)
                    nc.tensor.matmul(pp, wo2_sb[:, ts(m, 128)], yt2[:, qsl],
                                     start=False, stop=True)
                    ot = co_pool.tile([128, QCH], F32, tag="ot", name="ot")
                    nc.vector.tensor_copy(ot, pp)
                    nc.sync.dma_start(out=outT[ts(m, 128), qsl], in_=ot)

            def emit_body():
                # software-pipelined emission: proj_a leads post_a by 2
                # tiles, and 2 tiles of the NEXT chunk's projections are
                # emitted before each attention(qc) so the PE queue never
                # drains while the stage-A post chains complete.
                state = {"p": 0, "d": 0}

                def advance_to(need_post, proj_ahead):
                    cap = min(need_post + proj_ahead, NTT)
                    while state["d"] < need_post or state["p"] < cap:
                        if state["p"] < cap and state["p"] - state["d"] <= 2:
                            proj_a(state["p"])
                            state["p"] += 1
                        elif state["d"] < state["p"]:
                            post_a(state["d"])
                            state["d"] += 1
                        else:
                            break

                def mk(fn, i, key):
                    def f():
                        fn(i)
                        state[key] = i + 1
                    return f

                for qc in range(NQC):
                    advance_to(4 * qc + 4, 2)
                    # next-chunk stage-A work rides inside attention's pair
                    # loop, keeping both PE and DVE queues supplied
                    filler = []
                    plan_p, plan_d = state["p"], state["d"]
                    for _ in range(2):
                        if plan_p < min(NTT, 4 * qc + 8):
                            filler.append(mk(proj_a, plan_p, "p"))
                            plan_p += 1
                        if plan_d < min(4 * qc + 6, NTT):
                            filler.append(mk(post_a, plan_d, "d"))
                            plan_d += 1
                    fin = attention(qc, filler)
                    # two more full tiles between attention and cproj: their
                    # PE work covers the trailing normalize chains
                    advance_to(min(4 * qc + 8, NTT), 2)
                    cproj(qc, fin)

            if REPEAT > 1:
                with tc.For_i(0, REPEAT, 1):
                    emit_body()
            else:
                emit_body()

    nc.finalize()
    return nc


_NC = None


def _get_nc():
    global _NC
    if _NC is None:
        _NC = build_nc()
    return _NC


def _prep_inputs(x, wq, wk, wv, wo, alpha, cos, sin):
    x = np.asarray(x, dtype=np.float32)
    wq = np.asarray(wq, dtype=np.float32)
    wk = np.asarray(wk, dtype=np.float32)
    wv = np.asarray(wv, dtype=np.float32)
    wo = np.asarray(wo, dtype=np.float32)
    alpha = np.asarray(alpha, dtype=np.float32)
    cos = np.asarray(cos, dtype=np.float32)
    sin = np.asarray(sin, dtype=np.float32)

    # softmax over basis heads (fp32, stable)
    a = alpha - alpha.max(axis=-1, keepdims=True)
    e = np.exp(a)
    w = e / e.sum(axis=-1, keepdims=True)          # [H, KH]

    # fold the basis combination into effective per-head wk / wv
    wk_eff = np.einsum("dje,hj->dhe", wk.reshape(D, KH, HD), w).reshape(D, H * HD)
    wv_eff = np.einsum("dje,hj->dhe", wv.reshape(D, KH, HD), w).reshape(D, H * HD)

    csn = np.ascontiguousarray(np.concatenate([cos, sin], axis=1))     # [T, 64]
    scn = np.ascontiguousarray(np.concatenate([-sin, cos], axis=1))    # [T, 64]

    # single [128, 128] triangular mask (k <= q) for diagonal sub-blocks
    kk = np.arange(128)[:, None]
    qq = np.arange(128)[None, :]
    masks = np.ascontiguousarray((kk <= qq).astype(np.float32))

    import ml_dtypes
    bf16 = ml_dtypes.bfloat16

    in_maps = []
    for c in range(NCORES):
        b, g = c // 4, c % 4
        sl = slice(g * HPC * HD, (g + 1) * HPC * HD)
        wqkv = np.zeros((D, WQKV), dtype=np.float32)
        wqkv[:, 0:192] = wq[:, sl]
        wqkv[:, 192:384] = wk_eff[:, sl]
        wqkv[:, 384:576] = wv_eff[:, sl]
        wo_c = np.ascontiguousarray(wo[sl, :])
        in_maps.append({
            "xT": np.ascontiguousarray(x[b].T).astype(bf16),
            "wqkv": wqkv.astype(bf16),
            "wo01": np.ascontiguousarray(wo_c[0:128, :]).astype(bf16),
            "wo2": np.ascontiguousarray(wo_c[128:192, :]).astype(bf16),
            "csn": csn.astype(bf16),
            "scn": scn.astype(bf16),
            "masks": masks.astype(bf16),
        })
    return in_maps


def run(trace=False, **inputs):
    nc = _get_nc()
    in_maps = _prep_inputs(**inputs)
    res = run_bass_kernel_spmd(nc, in_maps, list(range(NCORES)), trace=trace)
    out = np.zeros((B, T, D), dtype=np.float32)
    for c in range(NCORES):
        out[c // 4] += res.results[c]["outT"].T
    return out, res


def kernel(**inputs):
    out, _ = run(**inputs)
    return out
